# revision 25
# baseline (speedup 1.0000x reference)
"""GAT (2-layer) on 8 NeuronCores — Bass/Tile kernel.

Strategy (dst-sharded graph parallel, host-expanded dense streams):
  - Each core owns 12500 destination nodes, degree-sorted into 128-dst
    tiles; tiles are paired and grouped into calls with a shared
    per-call slot capacity D (cross-core max), giving a dense
    [128 dst x D slot] layout per tile.
  - Launch A: per-core Wh1^T = (x W1 + b)^T and attention scalars
    s_i/s_j (all model FLOPs on device).
  - Host pre/re-pack (pure indexing of device-computed values): expands
    the per-edge source stream  stream16[p, col, f] = Wh[src] (fp16),
    sj_slot[p, col] = s_j[src] (f32, -1e30 at pad slots), si per tile,
    zero-degree flags.  No arithmetic on features happens on host.
  - Launch B (x2, one per GAT layer): streams the dense fp16 tables at
    line rate (plain dma_start, no gathers), computes masked segment
    softmax over the slot axis, alpha-weighted message sum (fp16
    multiply in place, f32 accumulate), leaky-relu, and the epilogue
    matmul with the next layer's weights (block-diagonal pair trick)
    -> next-layer Wh^T + attention scalars (or final fc output).
"""

import dataclasses
import numpy as np

import concourse.bacc as bacc
import concourse.tile as tile
from concourse import bass, mybir, bass_utils
from concourse.masks import make_identity

F32 = mybir.dt.float32
F16 = mybir.dt.float16

N_NODES = 100000
N_CORES = 8
DPC = N_NODES // N_CORES
F = 64
IN_C = 128
NEG_BIG = -1.0e30
ALPHA = 0.2
CALL_W = 256  # max slot-columns per call chunk
FLUSH_PAIRS = 4  # tile-pairs per epilogue matmul (512 psum cols)


@dataclasses.dataclass
class Schedule:
    n_tiles: int  # tiles per core (even)
    w_total: int  # total slot columns
    calls: list  # (t0, ntc, D, col0) ; ntc even
    gids: np.ndarray  # [N_CORES, n_tiles*128] global dst id or -1
    slot_src: np.ndarray  # [N_CORES, 128, w_total] src id or N_NODES (pad)
    si_gid: np.ndarray  # [N_CORES, 128, n_tiles] dst gid clipped (for si gather)
    flags: np.ndarray  # [N_CORES, 128, n_tiles] f32 1.0 where real dst with deg>0
    tile_col0: np.ndarray  # [n_tiles] starting col of each tile
    tile_D: np.ndarray  # [n_tiles] capacity of each tile


def build_schedule(edge_index: np.ndarray) -> Schedule:
    src = np.asarray(edge_index[0], dtype=np.int64)
    dst = np.asarray(edge_index[1], dtype=np.int64)
    order = np.argsort(dst, kind="stable")
    src_s = src[order]
    deg_all = np.bincount(dst, minlength=N_NODES).astype(np.int64)
    starts_all = np.concatenate([[0], np.cumsum(deg_all)])

    n_tiles = -(-DPC // 128)
    if n_tiles % 2:
        n_tiles += 1
    ntile_slots = n_tiles * 128

    # per-core degree-sorted dst order, padded with -1
    gids = np.full((N_CORES, ntile_slots), -1, np.int64)
    for c in range(N_CORES):
        degc = deg_all[c * DPC : (c + 1) * DPC]
        rank = np.argsort(degc, kind="stable")
        gids[c, :DPC] = c * DPC + rank

    deg_pad = np.concatenate([deg_all, [0]])
    gclip = np.where(gids >= 0, gids, N_NODES)
    degs = deg_pad[gclip].reshape(N_CORES, n_tiles, 128)
    tile_max = degs.max(axis=2).max(axis=0)  # [n_tiles] cross-core max deg

    # call plan over tile PAIRS: group pairs while ntc*D <= CALL_W
    pair_max = np.maximum(tile_max[0::2], tile_max[1::2])
    calls = []
    col = 0
    p0 = 0
    n_pairs = n_tiles // 2
    def rup4(x):
        return (int(x) + 3) // 4 * 4

    while p0 < n_pairs:
        D = max(4, rup4(pair_max[p0]))
        npair = 1
        while p0 + npair < n_pairs:
            nd = max(D, rup4(pair_max[p0 + npair]))
            if (npair + 1) * 2 * nd > CALL_W:
                break
            D = nd
            npair += 1
        calls.append((2 * p0, 2 * npair, D, col))
        col += 2 * npair * D
        p0 += npair
    w_total = col

    tile_col0 = np.zeros(n_tiles, np.int64)
    tile_D = np.zeros(n_tiles, np.int64)
    for (t0, ntc, D, col0) in calls:
        for tl in range(ntc):
            tile_col0[t0 + tl] = col0 + tl * D
            tile_D[t0 + tl] = D

    # slot_src: vectorized CSR -> padded-slot scatter
    slot_src = np.full((N_CORES, 128, w_total), N_NODES, np.int64)
    colstart_of_slot = tile_col0[
        np.arange(ntile_slots) // 128
    ]  # [ntile_slots] per (tile,partition)
    for c in range(N_CORES):
        g = gclip[c]
        ne = deg_pad[g]
        p_of_slot = np.arange(ntile_slots) % 128
        # flat positions in [128, w_total]: p*w_total + colstart + d
        base = p_of_slot * w_total + colstart_of_slot
        tot = int(ne.sum())
        pos = np.repeat(base, ne) + (
            np.arange(tot) - np.repeat(np.cumsum(ne) - ne, ne)
        )
        srcidx = np.repeat(starts_all[g], ne) + (
            np.arange(tot) - np.repeat(np.cumsum(ne) - ne, ne)
        )
        flat = slot_src[c].reshape(-1)
        flat[pos] = src_s[srcidx]

    si_gid = gclip.reshape(N_CORES, n_tiles, 128).transpose(0, 2, 1)
    flags = (
        ((gids >= 0) & (deg_pad[gclip] > 0))
        .reshape(N_CORES, n_tiles, 128)
        .transpose(0, 2, 1)
        .astype(np.float32)
    )
    flags = np.ascontiguousarray(flags)
    si_gid = np.ascontiguousarray(si_gid)

    return Schedule(
        n_tiles, w_total, calls, gids, slot_src, si_gid, flags, tile_col0, tile_D
    )


# ---------------------------------------------------------------- prog A
def build_progA(n_loc=DPC, in_c=IN_C, f=F):
    """whs[0:64] = (x W + bW)^T fp16 ; whs[64] = s_i ; whs[65] = s_j.

    Uses an augmented weight Waug = [W | W@A_i | W@A_j] (built on device)
    so each 512-column chunk is one matmul + one activation:
      x (W As) + bW As == ((x W + bW) As).
    """
    AF = mybir.ActivationFunctionType
    nc = bacc.Bacc("TRN2", target_bir_lowering=False, debug=False, num_devices=N_CORES)
    xT = nc.dram_tensor("xT", [in_c, n_loc], F16, kind="ExternalInput").ap()
    W = nc.dram_tensor("W", [in_c, f], F16, kind="ExternalInput").ap()
    bW = nc.dram_tensor("bW", [f, 1], F32, kind="ExternalInput").ap()
    As = nc.dram_tensor("As", [f, 2], F16, kind="ExternalInput").ap()
    whs = nc.dram_tensor("whs", [f + 2, n_loc], F16, kind="ExternalOutput").ap()

    CH = 512
    BATCH = 4

    with tile.TileContext(nc) as tc:
        with tc.tile_pool(name="sb", bufs=1) as pool, tc.tile_pool(
            name="ps", bufs=3, space="PSUM"
        ) as pps, tc.tile_pool(name="sb2", bufs=3) as pool2:
            xT_sb = pool.tile([in_c, n_loc], F16)
            nc.sync.dma_start(out=xT_sb[:], in_=xT[:, :])
            W_sb = pool.tile([in_c, f], F16)
            nc.sync.dma_start(out=W_sb[:], in_=W[:, :])
            bW_sb = pool.tile([f, 1], F32)
            nc.sync.dma_start(out=bW_sb[:], in_=bW[:, :])
            As_sb = pool.tile([f, 2], F16)
            nc.sync.dma_start(out=As_sb[:], in_=As[:, :])
            ident = pool.tile([128, 128], F16)
            make_identity(nc, ident[:])

            # Waug = [W | W@As] built on device
            Waug = pool.tile([in_c, f + 2], F16)
            nc.vector.tensor_copy(out=Waug[:, :f], in_=W_sb[:])
            ps_wt = pps.tile([f, 128], F16, space="PSUM", bufs=1)
            nc.tensor.transpose(out=ps_wt[:], in_=W_sb[:], identity=ident[:])
            WT_sb = pool.tile([f, 128], F16)
            nc.scalar.activation(out=WT_sb[:], in_=ps_wt[:], func=AF.Identity)
            ps_was = pps.tile([2, 128], F32, space="PSUM", bufs=1)
            nc.tensor.matmul(
                out=ps_was[:], lhsT=As_sb[:], rhs=WT_sb[:], start=True, stop=True
            )
            WAsT_sb = pool.tile([2, 128], F16)
            nc.scalar.activation(out=WAsT_sb[:], in_=ps_was[:], func=AF.Identity)
            ps_was2 = pps.tile([128, 2], F16, space="PSUM", bufs=1)
            nc.tensor.transpose(
                out=ps_was2[:], in_=WAsT_sb[:], identity=ident[:2, :2]
            )
            nc.scalar.activation(out=Waug[:, f : f + 2], in_=ps_was2[:], func=AF.Identity)

            # baug = [bW ; bW@As]
            baug = pool.tile([f + 2, 1], F32)
            nc.vector.tensor_copy(out=baug[:f], in_=bW_sb[:])
            bW16 = pool.tile([f, 1], F16)
            nc.vector.tensor_copy(out=bW16[:], in_=bW_sb[:])
            ps_bas = pps.tile([2, 1], F32, space="PSUM", bufs=1)
            nc.tensor.matmul(
                out=ps_bas[:], lhsT=As_sb[:], rhs=bW16[:], start=True, stop=True
            )
            nc.vector.tensor_copy(out=baug[f : f + 2], in_=ps_bas[:])

            for b0 in range(0, n_loc, CH * BATCH):
                bw = min(CH * BATCH, n_loc - b0)
                out_sb = pool2.tile([f + 2, CH * BATCH], F16, tag="out")
                for c0 in range(b0, b0 + bw, CH):
                    ch = min(CH, b0 + bw - c0)
                    ps_w = pps.tile([f + 2, CH], F32, tag="psw", space="PSUM")
                    nc.tensor.matmul(
                        out=ps_w[:, :ch],
                        lhsT=Waug[:],
                        rhs=xT_sb[:, c0 : c0 + ch],
                        start=True,
                        stop=True,
                    )
                    nc.scalar.activation(
                        out=out_sb[:, c0 - b0 : c0 - b0 + ch],
                        in_=ps_w[:, :ch],
                        func=AF.Identity,
                        bias=baug[:],
                    )
                nc.sync.dma_start(out=whs[:, b0 : b0 + bw], in_=out_sb[:, :bw])
    nc.compile()
    return nc


# ---------------------------------------------------------------- prog B
def build_progB(sched: Schedule, f=F):
    NT = sched.n_tiles
    WTOT = sched.w_total
    NPAIR = NT // 2
    nc = bacc.Bacc("TRN2", target_bir_lowering=False, debug=False, num_devices=N_CORES)
    stream = nc.dram_tensor("stream", [128, WTOT * f], F16, kind="ExternalInput").ap()
    sj_d = nc.dram_tensor("sj", [128, WTOT], F32, kind="ExternalInput").ap()
    si_d = nc.dram_tensor("si", [128, NT], F32, kind="ExternalInput").ap()
    flags_d = nc.dram_tensor("flags", [128, NT], F32, kind="ExternalInput").ap()
    bA_d = nc.dram_tensor("bA", [128, 1], F32, kind="ExternalInput").ap()
    WnBD_d = nc.dram_tensor("WnBD", [128, 128], F16, kind="ExternalInput").ap()
    bWn_d = nc.dram_tensor("bWn", [128, 1], F32, kind="ExternalInput").ap()
    AsBD_d = nc.dram_tensor("AsBD", [128, 4], F16, kind="ExternalInput").ap()
    whnT = nc.dram_tensor("whnT", [128, NPAIR * 128], F16, kind="ExternalOutput").ap()
    sn = nc.dram_tensor("sn", [4, NPAIR * 128], F32, kind="ExternalOutput").ap()

    X = mybir.AxisListType.X
    AF = mybir.ActivationFunctionType
    OP = mybir.AluOpType
    MAXNTC = max(ntc for (_, ntc, _, _) in sched.calls)

    def v(ap, dims, off=0):
        return dataclasses.replace(
            ap,
            ap=[list(ap.ap[0])] + [list(d) for d in dims],
            offset=ap.offset + off,
        )

    with tile.TileContext(nc) as tc:
        with tc.tile_pool(name="const", bufs=1) as pc, tc.tile_pool(
            name="io", bufs=3
        ) as pio, tc.tile_pool(name="work", bufs=2) as pw, tc.tile_pool(
            name="ps", bufs=2, space="PSUM"
        ) as pps, tc.tile_pool(name="ps2", bufs=2, space="PSUM") as pps2, tc.tile_pool(
            name="ep", bufs=2
        ) as pep:
            sj_sb = pc.tile([128, WTOT], F32)
            nc.sync.dma_start(out=sj_sb[:], in_=sj_d[:, :])
            si_sb = pc.tile([128, NT], F32)
            nc.sync.dma_start(out=si_sb[:], in_=si_d[:, :])
            flags_sb = pc.tile([128, NT], F32)
            nc.sync.dma_start(out=flags_sb[:], in_=flags_d[:, :])
            bA_sb = pc.tile([128, 1], F32)
            nc.sync.dma_start(out=bA_sb[:], in_=bA_d[:, :])
            WnBD_sb = pc.tile([128, 128], F16)
            nc.sync.dma_start(out=WnBD_sb[:], in_=WnBD_d[:, :])
            bWn_sb = pc.tile([128, 1], F32)
            nc.sync.dma_start(out=bWn_sb[:], in_=bWn_d[:, :])
            AsBD_sb = pc.tile([128, 4], F16)
            nc.sync.dma_start(out=AsBD_sb[:], in_=AsBD_d[:, :])
            ident = pc.tile([128, 128], F16)
            make_identity(nc, ident[:])

            # epilogue flush state: stacked-pair h columns awaiting matmul
            state = {"hgrp": None, "k0": 0, "n": 0}

            def flush_pairs():
                if not state["n"]:
                    return
                hgrp = state["hgrp"]
                k0 = state["k0"]
                cols = state["n"] * 128
                ps_w = pps2.tile([128, FLUSH_PAIRS * 128], F32, tag="psw", space="PSUM")
                nc.tensor.matmul(
                    out=ps_w[:, :cols],
                    lhsT=WnBD_sb[:],
                    rhs=hgrp[:, :cols],
                    start=True,
                    stop=True,
                )
                whn_sb = pep.tile([128, FLUSH_PAIRS * 128], F16, tag="whn")
                nc.scalar.activation(
                    out=whn_sb[:, :cols],
                    in_=ps_w[:, :cols],
                    func=AF.Identity,
                    bias=bWn_sb[:],
                )
                nc.sync.dma_start(
                    out=whnT[:, k0 * 128 : k0 * 128 + cols], in_=whn_sb[:, :cols]
                )
                ps_s = pps2.tile([4, FLUSH_PAIRS * 128], F32, tag="pss", space="PSUM")
                nc.tensor.matmul(
                    out=ps_s[:, :cols],
                    lhsT=AsBD_sb[:],
                    rhs=whn_sb[:, :cols],
                    start=True,
                    stop=True,
                )
                s_sb = pep.tile([4, FLUSH_PAIRS * 128], F32, tag="ssb")
                nc.scalar.activation(
                    out=s_sb[:, :cols], in_=ps_s[:, :cols], func=AF.Identity
                )
                nc.sync.dma_start(
                    out=sn[:, k0 * 128 : k0 * 128 + cols], in_=s_sb[:, :cols]
                )
                state["hgrp"] = None
                state["n"] = 0

            for (t0, ntc, D, col0) in sched.calls:
                W = ntc * D
                st = pio.tile([128, CALL_W * f], F16, tag="st")
                nc.sync.dma_start(
                    out=st[:, : W * f], in_=stream[:, col0 * f : (col0 + W) * f]
                )
                # e = leaky(s_j + s_i + bA); pads carry -1e30 inside sj
                epre = pw.tile([128, CALL_W], F32, tag="epre")
                nc.vector.tensor_tensor(
                    out=v(epre[:], [(D, ntc), (1, D)]),
                    in0=v(sj_sb[:], [(D, ntc), (1, D)], off=col0),
                    in1=si_sb[:, t0 : t0 + ntc].to_broadcast([128, ntc, D]),
                    op=OP.add,
                )
                e1 = pw.tile([128, CALL_W], F32, tag="e1")
                nc.scalar.activation(
                    out=e1[:, :W],
                    in_=epre[:, :W],
                    func=AF.Prelu,
                    bias=bA_sb[:],
                    alpha=ALPHA,
                )
                # segment softmax over slot axis. No max-subtraction: the
                # shift cancels in exp(e)/sum(exp(e)) and |e| <= ~20 here;
                # +1e-30 guards all-pad (phantom) rows against 1/0.
                ex = pw.tile([128, CALL_W], F32, tag="ex")
                nc.scalar.activation(out=ex[:, :W], in_=e1[:, :W], func=AF.Exp)
                den = pw.tile([128, MAXNTC], F32, tag="den")
                nc.vector.tensor_reduce(
                    out=den[:, :ntc],
                    in_=v(ex[:], [(D, ntc), (1, D)]),
                    axis=X,
                    op=OP.add,
                )
                dene = pw.tile([128, MAXNTC], F32, tag="dene")
                nc.vector.tensor_scalar(
                    out=dene[:, :ntc],
                    in0=den[:, :ntc],
                    scalar1=1e-30,
                    scalar2=None,
                    op0=OP.add,
                )
                rnorm = pw.tile([128, MAXNTC], F32, tag="rnorm")
                nc.vector.reciprocal(out=rnorm[:, :ntc], in_=dene[:, :ntc])
                nc.vector.tensor_tensor(
                    out=rnorm[:, :ntc],
                    in0=rnorm[:, :ntc],
                    in1=flags_sb[:, t0 : t0 + ntc],
                    op=OP.mult,
                )
                exn = pw.tile([128, CALL_W], F16, tag="exn")
                nc.vector.tensor_tensor(
                    out=v(exn[:], [(D, ntc), (1, D)]),
                    in0=v(ex[:], [(D, ntc), (1, D)]),
                    in1=rnorm[:, :ntc].to_broadcast([128, ntc, D]),
                    op=OP.mult,
                )
                # weighted messages in place over the stream tile (fp16,
                # feature-major: element (t, j, d) at offset t*f*D + j*D + d)
                nc.vector.tensor_tensor(
                    out=v(st[:], [(f * D, ntc), (D, f), (1, D)]),
                    in0=v(st[:], [(f * D, ntc), (D, f), (1, D)]),
                    in1=v(exn[:], [(D, ntc), (0, f), (1, D)]),
                    op=OP.mult,
                )
                # fold D -> D/2 with a 2x-mode tensor_tensor add (D % 4 == 0
                # so both halves stay pair-aligned), then 1x-mode reduce
                D2 = D // 2
                with nc.allow_low_precision(reason="fp16 segment sum, <=128 terms"):
                    nc.gpsimd.tensor_tensor(
                        out=v(st[:], [(f * D, ntc), (D, f), (1, D2)]),
                        in0=v(st[:], [(f * D, ntc), (D, f), (1, D2)]),
                        in1=v(st[:], [(f * D, ntc), (D, f), (1, D2)], off=D2),
                        op=OP.add,
                    )
                    hc = pw.tile([128, MAXNTC * f], F16, tag="hc")
                    nc.vector.tensor_reduce(
                        out=hc[:, : ntc * f],
                        in_=v(st[:], [(f * D, ntc), (D, f), (1, D2)]),
                        axis=X,
                        op=OP.add,
                    )
                # epilogue per tile pair: transpose + leaky into the flush group
                for pr in range(ntc // 2):
                    kpair = (t0 + 2 * pr) // 2
                    ps_t = pps.tile([128, 128], F16, tag="pst", space="PSUM")
                    nc.tensor.transpose(
                        out=ps_t[:],
                        in_=hc[:, 2 * pr * f : (2 * pr + 2) * f],
                        identity=ident[:],
                    )
                    if state["n"] == 0:
                        state["hgrp"] = pep.tile(
                            [128, FLUSH_PAIRS * 128], F16, tag="hgrp", name="hgrp"
                        )
                        state["k0"] = kpair
                    j = state["n"]
                    nc.scalar.activation(
                        out=state["hgrp"][:, j * 128 : (j + 1) * 128],
                        in_=ps_t[:],
                        func=AF.Prelu,
                        alpha=ALPHA,
                    )
                    state["n"] += 1
                    if state["n"] == FLUSH_PAIRS:
                        flush_pairs()
            flush_pairs()
    nc.compile()
    return nc


# ---------------------------------------------------------------- driver
_cache = {}


def kernel(x, edge_index, W1, bW1, A1, bA1, W2, bW2, A2, bA2, Wfc, bfc):
    x = np.asarray(x, dtype=np.float32)
    edge_index = np.asarray(edge_index)
    W1 = np.asarray(W1, np.float32)
    bW1 = np.asarray(bW1, np.float32)
    A1 = np.asarray(A1, np.float32)
    bA1 = np.asarray(bA1, np.float32)
    W2 = np.asarray(W2, np.float32)
    bW2 = np.asarray(bW2, np.float32)
    A2 = np.asarray(A2, np.float32)
    bA2 = np.asarray(bA2, np.float32)
    Wfc = np.asarray(Wfc, np.float32)
    bfc = np.asarray(bfc, np.float32)

    sched = build_schedule(edge_index)
    cores = list(range(N_CORES))
    NT = sched.n_tiles
    NPAIR = NT // 2

    if "A" not in _cache:
        _cache["A"] = build_progA()
    ncA = _cache["A"]
    inA = []
    x16T = np.ascontiguousarray(x.T.astype(np.float16))
    W1_16 = W1.astype(np.float16)
    As1_16 = np.ascontiguousarray(
        np.concatenate([A1[:F], A1[F:]], axis=1).astype(np.float16)
    )
    for c in cores:
        inA.append(
            {
                "xT": np.ascontiguousarray(x16T[:, c * DPC : (c + 1) * DPC]),
                "W": W1_16,
                "bW": bW1.reshape(F, 1),
                "As": As1_16,
            }
        )
    resA = bass_utils.run_bass_kernel_spmd(ncA, inA, core_ids=cores)
    whs = np.concatenate([resA.results[c]["whs"] for c in cores], axis=1)
    wh = np.ascontiguousarray(whs[:F].T)
    si_full = whs[F].astype(np.float32)
    sj_full = whs[F + 1].astype(np.float32)

    key = ("B", NT, sched.w_total, tuple(sched.calls))
    if key not in _cache:
        _cache[key] = build_progB(sched)
    ncB = _cache[key]

    def launch_B(wh_full, si_f, sj_f, bA, Wn, bWn, An):
        wh16 = np.concatenate(
            [wh_full.astype(np.float16), np.zeros((1, F), np.float16)], axis=0
        )
        sjpad = np.concatenate([sj_f, [np.float32(NEG_BIG)]]).astype(np.float32)
        sipad = np.concatenate([si_f, [np.float32(0.0)]]).astype(np.float32)
        WnBD = np.zeros((128, 128), np.float16)
        WnBD[:F, :F] = Wn
        WnBD[F:, F:] = Wn
        AsBD = np.zeros((128, 4), np.float16)
        AsBD[:F, 0:1] = An[:, 0:1]
        AsBD[:F, 1:2] = An[:, 1:2]
        AsBD[F:, 2:3] = An[:, 0:1]
        AsBD[F:, 3:4] = An[:, 1:2]
        bWn2 = np.concatenate([bWn.reshape(F), bWn.reshape(F)]).reshape(128, 1)
        inB = []
        for c in cores:
            ss = sched.slot_src[c]
            # feature-major stream: per call, element (t, j, d) at t*F*D+j*D+d
            stream = np.empty((128, sched.w_total * F), np.float16)
            for (t0, ntc, D, col0) in sched.calls:
                W = ntc * D
                blk = wh16[ss[:, col0 : col0 + W]].reshape(128, ntc, D, F)
                stream[:, col0 * F : (col0 + W) * F] = (
                    blk.transpose(0, 1, 3, 2).reshape(128, W * F)
                )
            inB.append(
                {
                    "stream": stream,
                    "sj": sjpad[ss],
                    "si": sipad[sched.si_gid[c]],
                    "flags": sched.flags[c],
                    "bA": np.full((128, 1), bA.reshape(-1)[0], np.float32),
                    "WnBD": WnBD,
                    "bWn": bWn2,
                    "AsBD": AsBD,
                }
            )
        res = bass_utils.run_bass_kernel_spmd(ncB, inB, core_ids=cores)
        whn = np.zeros((N_NODES, F), np.float32)
        sn_i = np.zeros(N_NODES, np.float32)
        sn_j = np.zeros(N_NODES, np.float32)
        for c in cores:
            gids = sched.gids[c]
            real = gids >= 0
            w = res.results[c]["whnT"].astype(np.float32).reshape(128, NPAIR, 128)
            snc = res.results[c]["sn"].reshape(4, NPAIR, 128)
            # tile 2k -> rows 0:64 of pair k; tile 2k+1 -> rows 64:128
            wA = w[:F].transpose(1, 2, 0)  # [NPAIR, 128, F] even tiles
            wB = w[F:].transpose(1, 2, 0)  # odd tiles
            wfull = np.empty((NT, 128, F), np.float32)
            wfull[0::2] = wA
            wfull[1::2] = wB
            sfull_i = np.empty((NT, 128), np.float32)
            sfull_j = np.empty((NT, 128), np.float32)
            sfull_i[0::2] = snc[0]
            sfull_i[1::2] = snc[2]
            sfull_j[0::2] = snc[1]
            sfull_j[1::2] = snc[3]
            whn[gids[real]] = wfull.reshape(NT * 128, F)[real]
            sn_i[gids[real]] = sfull_i.reshape(-1)[real]
            sn_j[gids[real]] = sfull_j.reshape(-1)[real]
        return whn, sn_i, sn_j

    As2 = np.ascontiguousarray(np.concatenate([A2[:F], A2[F:]], axis=1))
    wh2, si2, sj2 = launch_B(wh, si_full, sj_full, bA1, W2, bW2, As2)
    out, _, _ = launch_B(wh2, si2, sj2, bA2, Wfc, bfc, np.zeros((F, 2), np.float32))
    return out.astype(np.float32)


# revision 26
# speedup vs baseline: 1.4177x; 1.4177x over previous
"""GAT (2-layer) on 8 NeuronCores — Bass/Tile kernel.

Strategy (dst-sharded graph parallel, host-expanded dense streams):
  - Each core owns 12500 destination nodes, degree-sorted into 128-dst
    tiles; tiles are paired and grouped into calls with a shared
    per-call slot capacity D (cross-core max), giving a dense
    [128 dst x D slot] layout per tile.
  - Launch A: per-core Wh1^T = (x W1 + b)^T and attention scalars
    s_i/s_j (all model FLOPs on device).
  - Host pre/re-pack (pure indexing of device-computed values): expands
    the per-edge source stream  stream16[p, col, f] = Wh[src] (fp16),
    sj_slot[p, col] = s_j[src] (f32, -1e30 at pad slots), si per tile,
    zero-degree flags.  No arithmetic on features happens on host.
  - Launch B (x2, one per GAT layer): streams the dense fp16 tables at
    line rate (plain dma_start, no gathers), computes masked segment
    softmax over the slot axis, alpha-weighted message sum (fp16
    multiply in place, f32 accumulate), leaky-relu, and the epilogue
    matmul with the next layer's weights (block-diagonal pair trick)
    -> next-layer Wh^T + attention scalars (or final fc output).
"""

import dataclasses
import numpy as np

import concourse.bacc as bacc
import concourse.tile as tile
from concourse import bass, mybir, bass_utils
from concourse.masks import make_identity

F32 = mybir.dt.float32
F16 = mybir.dt.float16

N_NODES = 100000
N_CORES = 8
DPC = N_NODES // N_CORES
F = 64
IN_C = 128
NEG_BIG = -1.0e30
ALPHA = 0.2
CALL_W = 256  # max slot-columns per call chunk
FLUSH_PAIRS = 4  # tile-pairs per epilogue matmul (512 psum cols)


@dataclasses.dataclass
class Schedule:
    n_tiles: int  # tiles per core (even)
    w_total: int  # total slot columns
    calls: list  # (t0, ntc, D, col0) ; ntc even
    gids: np.ndarray  # [N_CORES, n_tiles*128] global dst id or -1
    slot_src: np.ndarray  # [N_CORES, 128, w_total] src id or N_NODES (pad)
    si_gid: np.ndarray  # [N_CORES, 128, n_tiles] dst gid clipped (for si gather)
    flags: np.ndarray  # [N_CORES, 128, n_tiles] f32 1.0 where real dst with deg>0
    tile_col0: np.ndarray  # [n_tiles] starting col of each tile
    tile_D: np.ndarray  # [n_tiles] capacity of each tile


def build_schedule(edge_index: np.ndarray) -> Schedule:
    src = np.asarray(edge_index[0], dtype=np.int64)
    dst = np.asarray(edge_index[1], dtype=np.int64)
    order = np.argsort(dst, kind="stable")
    src_s = src[order]
    deg_all = np.bincount(dst, minlength=N_NODES).astype(np.int64)
    starts_all = np.concatenate([[0], np.cumsum(deg_all)])

    n_tiles = -(-DPC // 128)
    if n_tiles % 2:
        n_tiles += 1
    ntile_slots = n_tiles * 128

    # per-core degree-sorted dst order, padded with -1
    gids = np.full((N_CORES, ntile_slots), -1, np.int64)
    for c in range(N_CORES):
        degc = deg_all[c * DPC : (c + 1) * DPC]
        rank = np.argsort(degc, kind="stable")
        gids[c, :DPC] = c * DPC + rank

    deg_pad = np.concatenate([deg_all, [0]])
    gclip = np.where(gids >= 0, gids, N_NODES)
    degs = deg_pad[gclip].reshape(N_CORES, n_tiles, 128)
    tile_max = degs.max(axis=2).max(axis=0)  # [n_tiles] cross-core max deg

    # call plan over tile PAIRS: group pairs while ntc*D <= CALL_W
    pair_max = np.maximum(tile_max[0::2], tile_max[1::2])
    calls = []
    col = 0
    p0 = 0
    n_pairs = n_tiles // 2
    def rup4(x):
        return (int(x) + 3) // 4 * 4

    while p0 < n_pairs:
        D = max(4, rup4(pair_max[p0]))
        npair = 1
        while p0 + npair < n_pairs:
            nd = max(D, rup4(pair_max[p0 + npair]))
            if (npair + 1) * 2 * nd > CALL_W:
                break
            D = nd
            npair += 1
        calls.append((2 * p0, 2 * npair, D, col))
        col += 2 * npair * D
        p0 += npair
    w_total = col

    tile_col0 = np.zeros(n_tiles, np.int64)
    tile_D = np.zeros(n_tiles, np.int64)
    for (t0, ntc, D, col0) in calls:
        for tl in range(ntc):
            tile_col0[t0 + tl] = col0 + tl * D
            tile_D[t0 + tl] = D

    # slot_src: vectorized CSR -> padded-slot scatter
    slot_src = np.full((N_CORES, 128, w_total), N_NODES, np.int64)
    colstart_of_slot = tile_col0[
        np.arange(ntile_slots) // 128
    ]  # [ntile_slots] per (tile,partition)
    for c in range(N_CORES):
        g = gclip[c]
        ne = deg_pad[g]
        p_of_slot = np.arange(ntile_slots) % 128
        # flat positions in [128, w_total]: p*w_total + colstart + d
        base = p_of_slot * w_total + colstart_of_slot
        tot = int(ne.sum())
        pos = np.repeat(base, ne) + (
            np.arange(tot) - np.repeat(np.cumsum(ne) - ne, ne)
        )
        srcidx = np.repeat(starts_all[g], ne) + (
            np.arange(tot) - np.repeat(np.cumsum(ne) - ne, ne)
        )
        flat = slot_src[c].reshape(-1)
        flat[pos] = src_s[srcidx]

    si_gid = gclip.reshape(N_CORES, n_tiles, 128).transpose(0, 2, 1)
    flags = (
        ((gids >= 0) & (deg_pad[gclip] > 0))
        .reshape(N_CORES, n_tiles, 128)
        .transpose(0, 2, 1)
        .astype(np.float32)
    )
    flags = np.ascontiguousarray(flags)
    si_gid = np.ascontiguousarray(si_gid)

    return Schedule(
        n_tiles, w_total, calls, gids, slot_src, si_gid, flags, tile_col0, tile_D
    )


# ---------------------------------------------------------------- prog A
def build_progA(n_loc=DPC, in_c=IN_C, f=F):
    """whs[0:64] = (x W + bW)^T fp16 ; whs[64] = s_i ; whs[65] = s_j.

    Uses an augmented weight Waug = [W | W@A_i | W@A_j] (built on device)
    so each 512-column chunk is one matmul + one activation:
      x (W As) + bW As == ((x W + bW) As).
    """
    AF = mybir.ActivationFunctionType
    nc = bacc.Bacc("TRN2", target_bir_lowering=False, debug=False, num_devices=N_CORES)
    xT = nc.dram_tensor("xT", [in_c, n_loc], F16, kind="ExternalInput").ap()
    W = nc.dram_tensor("W", [in_c, f], F16, kind="ExternalInput").ap()
    bW = nc.dram_tensor("bW", [f, 1], F32, kind="ExternalInput").ap()
    As = nc.dram_tensor("As", [f, 2], F16, kind="ExternalInput").ap()
    whs = nc.dram_tensor("whs", [f + 2, n_loc], F16, kind="ExternalOutput").ap()

    CH = 512
    BATCH = 4

    with tile.TileContext(nc) as tc:
        with tc.tile_pool(name="sb", bufs=1) as pool, tc.tile_pool(
            name="ps", bufs=3, space="PSUM"
        ) as pps, tc.tile_pool(name="sb2", bufs=3) as pool2:
            xT_sb = pool.tile([in_c, n_loc], F16)
            nc.sync.dma_start(out=xT_sb[:], in_=xT[:, :])
            W_sb = pool.tile([in_c, f], F16)
            nc.sync.dma_start(out=W_sb[:], in_=W[:, :])
            bW_sb = pool.tile([f, 1], F32)
            nc.sync.dma_start(out=bW_sb[:], in_=bW[:, :])
            As_sb = pool.tile([f, 2], F16)
            nc.sync.dma_start(out=As_sb[:], in_=As[:, :])
            ident = pool.tile([128, 128], F16)
            make_identity(nc, ident[:])

            # Waug = [W | W@As] built on device
            Waug = pool.tile([in_c, f + 2], F16)
            nc.vector.tensor_copy(out=Waug[:, :f], in_=W_sb[:])
            ps_wt = pps.tile([f, 128], F16, space="PSUM", bufs=1)
            nc.tensor.transpose(out=ps_wt[:], in_=W_sb[:], identity=ident[:])
            WT_sb = pool.tile([f, 128], F16)
            nc.scalar.activation(out=WT_sb[:], in_=ps_wt[:], func=AF.Identity)
            ps_was = pps.tile([2, 128], F32, space="PSUM", bufs=1)
            nc.tensor.matmul(
                out=ps_was[:], lhsT=As_sb[:], rhs=WT_sb[:], start=True, stop=True
            )
            WAsT_sb = pool.tile([2, 128], F16)
            nc.scalar.activation(out=WAsT_sb[:], in_=ps_was[:], func=AF.Identity)
            ps_was2 = pps.tile([128, 2], F16, space="PSUM", bufs=1)
            nc.tensor.transpose(
                out=ps_was2[:], in_=WAsT_sb[:], identity=ident[:2, :2]
            )
            nc.scalar.activation(out=Waug[:, f : f + 2], in_=ps_was2[:], func=AF.Identity)

            # baug = [bW ; bW@As]
            baug = pool.tile([f + 2, 1], F32)
            nc.vector.tensor_copy(out=baug[:f], in_=bW_sb[:])
            bW16 = pool.tile([f, 1], F16)
            nc.vector.tensor_copy(out=bW16[:], in_=bW_sb[:])
            ps_bas = pps.tile([2, 1], F32, space="PSUM", bufs=1)
            nc.tensor.matmul(
                out=ps_bas[:], lhsT=As_sb[:], rhs=bW16[:], start=True, stop=True
            )
            nc.vector.tensor_copy(out=baug[f : f + 2], in_=ps_bas[:])

            for b0 in range(0, n_loc, CH * BATCH):
                bw = min(CH * BATCH, n_loc - b0)
                out_sb = pool2.tile([f + 2, CH * BATCH], F16, tag="out")
                for c0 in range(b0, b0 + bw, CH):
                    ch = min(CH, b0 + bw - c0)
                    ps_w = pps.tile([f + 2, CH], F32, tag="psw", space="PSUM")
                    nc.tensor.matmul(
                        out=ps_w[:, :ch],
                        lhsT=Waug[:],
                        rhs=xT_sb[:, c0 : c0 + ch],
                        start=True,
                        stop=True,
                    )
                    nc.scalar.activation(
                        out=out_sb[:, c0 - b0 : c0 - b0 + ch],
                        in_=ps_w[:, :ch],
                        func=AF.Identity,
                        bias=baug[:],
                    )
                nc.sync.dma_start(out=whs[:, b0 : b0 + bw], in_=out_sb[:, :bw])
    nc.compile()
    return nc


# ---------------------------------------------------------------- prog B
def build_progB(sched: Schedule, f=F):
    NT = sched.n_tiles
    WTOT = sched.w_total
    NPAIR = NT // 2
    nc = bacc.Bacc("TRN2", target_bir_lowering=False, debug=False, num_devices=N_CORES)
    stream = nc.dram_tensor("stream", [128, WTOT * f], F16, kind="ExternalInput").ap()
    sj_d = nc.dram_tensor("sj", [128, WTOT], F32, kind="ExternalInput").ap()
    si_d = nc.dram_tensor("si", [128, NT], F32, kind="ExternalInput").ap()
    flags_d = nc.dram_tensor("flags", [128, NT], F32, kind="ExternalInput").ap()
    bA_d = nc.dram_tensor("bA", [128, 1], F32, kind="ExternalInput").ap()
    WnBD_d = nc.dram_tensor("WnBD", [128, 128], F16, kind="ExternalInput").ap()
    bWn_d = nc.dram_tensor("bWn", [128, 1], F32, kind="ExternalInput").ap()
    AsBD_d = nc.dram_tensor("AsBD", [128, 4], F16, kind="ExternalInput").ap()
    whnT = nc.dram_tensor("whnT", [128, NPAIR * 128], F16, kind="ExternalOutput").ap()
    sn = nc.dram_tensor("sn", [4, NPAIR * 128], F32, kind="ExternalOutput").ap()

    X = mybir.AxisListType.X
    AF = mybir.ActivationFunctionType
    OP = mybir.AluOpType
    MAXNTC = max(ntc for (_, ntc, _, _) in sched.calls)

    def v(ap, dims, off=0):
        return dataclasses.replace(
            ap,
            ap=[list(ap.ap[0])] + [list(d) for d in dims],
            offset=ap.offset + off,
        )

    with tile.TileContext(nc) as tc:
        with tc.tile_pool(name="const", bufs=1) as pc, tc.tile_pool(
            name="io", bufs=3
        ) as pio, tc.tile_pool(name="work", bufs=2) as pw, tc.tile_pool(
            name="ps", bufs=2, space="PSUM"
        ) as pps, tc.tile_pool(name="ps2", bufs=2, space="PSUM") as pps2, tc.tile_pool(
            name="ep", bufs=2
        ) as pep:
            sj_sb = pc.tile([128, WTOT], F32)
            nc.sync.dma_start(out=sj_sb[:], in_=sj_d[:, :])
            si_sb = pc.tile([128, NT], F32)
            nc.sync.dma_start(out=si_sb[:], in_=si_d[:, :])
            flags_sb = pc.tile([128, NT], F32)
            nc.sync.dma_start(out=flags_sb[:], in_=flags_d[:, :])
            bA_sb = pc.tile([128, 1], F32)
            nc.sync.dma_start(out=bA_sb[:], in_=bA_d[:, :])
            WnBD_sb = pc.tile([128, 128], F16)
            nc.sync.dma_start(out=WnBD_sb[:], in_=WnBD_d[:, :])
            bWn_sb = pc.tile([128, 1], F32)
            nc.sync.dma_start(out=bWn_sb[:], in_=bWn_d[:, :])
            AsBD_sb = pc.tile([128, 4], F16)
            nc.sync.dma_start(out=AsBD_sb[:], in_=AsBD_d[:, :])
            ident = pc.tile([128, 128], F16)
            make_identity(nc, ident[:])

            # epilogue flush state: stacked-pair h columns awaiting matmul
            state = {"hgrp": None, "k0": 0, "n": 0}

            def flush_pairs():
                if not state["n"]:
                    return
                hgrp = state["hgrp"]
                k0 = state["k0"]
                cols = state["n"] * 128
                ps_w = pps2.tile([128, FLUSH_PAIRS * 128], F32, tag="psw", space="PSUM")
                nc.tensor.matmul(
                    out=ps_w[:, :cols],
                    lhsT=WnBD_sb[:],
                    rhs=hgrp[:, :cols],
                    start=True,
                    stop=True,
                )
                whn_sb = pep.tile([128, FLUSH_PAIRS * 128], F16, tag="whn")
                nc.scalar.activation(
                    out=whn_sb[:, :cols],
                    in_=ps_w[:, :cols],
                    func=AF.Identity,
                    bias=bWn_sb[:],
                )
                nc.sync.dma_start(
                    out=whnT[:, k0 * 128 : k0 * 128 + cols], in_=whn_sb[:, :cols]
                )
                ps_s = pps2.tile([4, FLUSH_PAIRS * 128], F32, tag="pss", space="PSUM")
                nc.tensor.matmul(
                    out=ps_s[:, :cols],
                    lhsT=AsBD_sb[:],
                    rhs=whn_sb[:, :cols],
                    start=True,
                    stop=True,
                )
                s_sb = pep.tile([4, FLUSH_PAIRS * 128], F32, tag="ssb")
                nc.scalar.activation(
                    out=s_sb[:, :cols], in_=ps_s[:, :cols], func=AF.Identity
                )
                nc.sync.dma_start(
                    out=sn[:, k0 * 128 : k0 * 128 + cols], in_=s_sb[:, :cols]
                )
                state["hgrp"] = None
                state["n"] = 0

            for (t0, ntc, D, col0) in sched.calls:
                W = ntc * D
                st = pio.tile([128, CALL_W * f], F16, tag="st")
                nc.sync.dma_start(
                    out=st[:, : W * f], in_=stream[:, col0 * f : (col0 + W) * f]
                )
                # e = leaky(s_j + s_i + bA); pads carry -1e30 inside sj
                epre = pw.tile([128, CALL_W], F32, tag="epre")
                nc.vector.tensor_tensor(
                    out=v(epre[:], [(D, ntc), (1, D)]),
                    in0=v(sj_sb[:], [(D, ntc), (1, D)], off=col0),
                    in1=si_sb[:, t0 : t0 + ntc].to_broadcast([128, ntc, D]),
                    op=OP.add,
                )
                e1 = pw.tile([128, CALL_W], F32, tag="e1")
                nc.scalar.activation(
                    out=e1[:, :W],
                    in_=epre[:, :W],
                    func=AF.Prelu,
                    bias=bA_sb[:],
                    alpha=ALPHA,
                )
                # segment softmax over slot axis. No max-subtraction: the
                # shift cancels in exp(e)/sum(exp(e)) and |e| <= ~20 here;
                # +1e-30 guards all-pad (phantom) rows against 1/0.
                ex = pw.tile([128, CALL_W], F32, tag="ex")
                nc.scalar.activation(out=ex[:, :W], in_=e1[:, :W], func=AF.Exp)
                den = pw.tile([128, MAXNTC], F32, tag="den")
                nc.vector.tensor_reduce(
                    out=den[:, :ntc],
                    in_=v(ex[:], [(D, ntc), (1, D)]),
                    axis=X,
                    op=OP.add,
                )
                dene = pw.tile([128, MAXNTC], F32, tag="dene")
                nc.vector.tensor_scalar(
                    out=dene[:, :ntc],
                    in0=den[:, :ntc],
                    scalar1=1e-30,
                    scalar2=None,
                    op0=OP.add,
                )
                rnorm = pw.tile([128, MAXNTC], F32, tag="rnorm")
                nc.vector.reciprocal(out=rnorm[:, :ntc], in_=dene[:, :ntc])
                nc.vector.tensor_tensor(
                    out=rnorm[:, :ntc],
                    in0=rnorm[:, :ntc],
                    in1=flags_sb[:, t0 : t0 + ntc],
                    op=OP.mult,
                )
                exn = pw.tile([128, CALL_W], F16, tag="exn")
                nc.vector.tensor_tensor(
                    out=v(exn[:], [(D, ntc), (1, D)]),
                    in0=v(ex[:], [(D, ntc), (1, D)]),
                    in1=rnorm[:, :ntc].to_broadcast([128, ntc, D]),
                    op=OP.mult,
                )
                # weighted messages in place over the stream tile (fp16,
                # feature-major: element (t, j, d) at offset t*f*D + j*D + d)
                nc.vector.tensor_tensor(
                    out=v(st[:], [(f * D, ntc), (D, f), (1, D)]),
                    in0=v(st[:], [(f * D, ntc), (D, f), (1, D)]),
                    in1=v(exn[:], [(D, ntc), (0, f), (1, D)]),
                    op=OP.mult,
                )
                # fold D -> D/2 with a 2x-mode tensor_tensor add (D % 4 == 0
                # so both halves stay pair-aligned), then 1x-mode reduce
                D2 = D // 2
                with nc.allow_low_precision(reason="fp16 segment sum, <=128 terms"):
                    nc.vector.tensor_tensor(
                        out=v(st[:], [(f * D, ntc), (D, f), (1, D2)]),
                        in0=v(st[:], [(f * D, ntc), (D, f), (1, D2)]),
                        in1=v(st[:], [(f * D, ntc), (D, f), (1, D2)], off=D2),
                        op=OP.add,
                    )
                    hc = pw.tile([128, MAXNTC * f], F16, tag="hc")
                    nc.vector.tensor_reduce(
                        out=hc[:, : ntc * f],
                        in_=v(st[:], [(f * D, ntc), (D, f), (1, D2)]),
                        axis=X,
                        op=OP.add,
                    )
                # epilogue per tile pair: transpose + leaky into the flush group
                for pr in range(ntc // 2):
                    kpair = (t0 + 2 * pr) // 2
                    ps_t = pps.tile([128, 128], F16, tag="pst", space="PSUM")
                    nc.tensor.transpose(
                        out=ps_t[:],
                        in_=hc[:, 2 * pr * f : (2 * pr + 2) * f],
                        identity=ident[:],
                    )
                    if state["n"] == 0:
                        state["hgrp"] = pep.tile(
                            [128, FLUSH_PAIRS * 128], F16, tag="hgrp", name="hgrp"
                        )
                        state["k0"] = kpair
                    j = state["n"]
                    nc.scalar.activation(
                        out=state["hgrp"][:, j * 128 : (j + 1) * 128],
                        in_=ps_t[:],
                        func=AF.Prelu,
                        alpha=ALPHA,
                    )
                    state["n"] += 1
                    if state["n"] == FLUSH_PAIRS:
                        flush_pairs()
            flush_pairs()
    nc.compile()
    return nc


# ---------------------------------------------------------------- driver
_cache = {}


def kernel(x, edge_index, W1, bW1, A1, bA1, W2, bW2, A2, bA2, Wfc, bfc):
    x = np.asarray(x, dtype=np.float32)
    edge_index = np.asarray(edge_index)
    W1 = np.asarray(W1, np.float32)
    bW1 = np.asarray(bW1, np.float32)
    A1 = np.asarray(A1, np.float32)
    bA1 = np.asarray(bA1, np.float32)
    W2 = np.asarray(W2, np.float32)
    bW2 = np.asarray(bW2, np.float32)
    A2 = np.asarray(A2, np.float32)
    bA2 = np.asarray(bA2, np.float32)
    Wfc = np.asarray(Wfc, np.float32)
    bfc = np.asarray(bfc, np.float32)

    sched = build_schedule(edge_index)
    cores = list(range(N_CORES))
    NT = sched.n_tiles
    NPAIR = NT // 2

    if "A" not in _cache:
        _cache["A"] = build_progA()
    ncA = _cache["A"]
    inA = []
    x16T = np.ascontiguousarray(x.T.astype(np.float16))
    W1_16 = W1.astype(np.float16)
    As1_16 = np.ascontiguousarray(
        np.concatenate([A1[:F], A1[F:]], axis=1).astype(np.float16)
    )
    for c in cores:
        inA.append(
            {
                "xT": np.ascontiguousarray(x16T[:, c * DPC : (c + 1) * DPC]),
                "W": W1_16,
                "bW": bW1.reshape(F, 1),
                "As": As1_16,
            }
        )
    resA = bass_utils.run_bass_kernel_spmd(ncA, inA, core_ids=cores)
    whs = np.concatenate([resA.results[c]["whs"] for c in cores], axis=1)
    wh = np.ascontiguousarray(whs[:F].T)
    si_full = whs[F].astype(np.float32)
    sj_full = whs[F + 1].astype(np.float32)

    key = ("B", NT, sched.w_total, tuple(sched.calls))
    if key not in _cache:
        _cache[key] = build_progB(sched)
    ncB = _cache[key]

    def launch_B(wh_full, si_f, sj_f, bA, Wn, bWn, An):
        wh16 = np.concatenate(
            [wh_full.astype(np.float16), np.zeros((1, F), np.float16)], axis=0
        )
        sjpad = np.concatenate([sj_f, [np.float32(NEG_BIG)]]).astype(np.float32)
        sipad = np.concatenate([si_f, [np.float32(0.0)]]).astype(np.float32)
        WnBD = np.zeros((128, 128), np.float16)
        WnBD[:F, :F] = Wn
        WnBD[F:, F:] = Wn
        AsBD = np.zeros((128, 4), np.float16)
        AsBD[:F, 0:1] = An[:, 0:1]
        AsBD[:F, 1:2] = An[:, 1:2]
        AsBD[F:, 2:3] = An[:, 0:1]
        AsBD[F:, 3:4] = An[:, 1:2]
        bWn2 = np.concatenate([bWn.reshape(F), bWn.reshape(F)]).reshape(128, 1)
        inB = []
        for c in cores:
            ss = sched.slot_src[c]
            # feature-major stream: per call, element (t, j, d) at t*F*D+j*D+d
            stream = np.empty((128, sched.w_total * F), np.float16)
            for (t0, ntc, D, col0) in sched.calls:
                W = ntc * D
                blk = wh16[ss[:, col0 : col0 + W]].reshape(128, ntc, D, F)
                stream[:, col0 * F : (col0 + W) * F] = (
                    blk.transpose(0, 1, 3, 2).reshape(128, W * F)
                )
            inB.append(
                {
                    "stream": stream,
                    "sj": sjpad[ss],
                    "si": sipad[sched.si_gid[c]],
                    "flags": sched.flags[c],
                    "bA": np.full((128, 1), bA.reshape(-1)[0], np.float32),
                    "WnBD": WnBD,
                    "bWn": bWn2,
                    "AsBD": AsBD,
                }
            )
        res = bass_utils.run_bass_kernel_spmd(ncB, inB, core_ids=cores)
        whn = np.zeros((N_NODES, F), np.float32)
        sn_i = np.zeros(N_NODES, np.float32)
        sn_j = np.zeros(N_NODES, np.float32)
        for c in cores:
            gids = sched.gids[c]
            real = gids >= 0
            w = res.results[c]["whnT"].astype(np.float32).reshape(128, NPAIR, 128)
            snc = res.results[c]["sn"].reshape(4, NPAIR, 128)
            # tile 2k -> rows 0:64 of pair k; tile 2k+1 -> rows 64:128
            wA = w[:F].transpose(1, 2, 0)  # [NPAIR, 128, F] even tiles
            wB = w[F:].transpose(1, 2, 0)  # odd tiles
            wfull = np.empty((NT, 128, F), np.float32)
            wfull[0::2] = wA
            wfull[1::2] = wB
            sfull_i = np.empty((NT, 128), np.float32)
            sfull_j = np.empty((NT, 128), np.float32)
            sfull_i[0::2] = snc[0]
            sfull_i[1::2] = snc[2]
            sfull_j[0::2] = snc[1]
            sfull_j[1::2] = snc[3]
            whn[gids[real]] = wfull.reshape(NT * 128, F)[real]
            sn_i[gids[real]] = sfull_i.reshape(-1)[real]
            sn_j[gids[real]] = sfull_j.reshape(-1)[real]
        return whn, sn_i, sn_j

    As2 = np.ascontiguousarray(np.concatenate([A2[:F], A2[F:]], axis=1))
    wh2, si2, sj2 = launch_B(wh, si_full, sj_full, bA1, W2, bW2, As2)
    out, _, _ = launch_B(wh2, si2, sj2, bA2, Wfc, bfc, np.zeros((F, 2), np.float32))
    return out.astype(np.float32)


# revision 27
# speedup vs baseline: 1.4577x; 1.0282x over previous
"""GAT (2-layer) on 8 NeuronCores — Bass/Tile kernel.

Strategy (dst-sharded graph parallel, host-expanded dense streams):
  - Each core owns 12500 destination nodes, degree-sorted into 128-dst
    tiles; tiles are paired and grouped into calls with a shared
    per-call slot capacity D (cross-core max), giving a dense
    [128 dst x D slot] layout per tile.
  - Launch A: per-core Wh1^T = (x W1 + b)^T and attention scalars
    s_i/s_j (all model FLOPs on device).
  - Host pre/re-pack (pure indexing of device-computed values): expands
    the per-edge source stream  stream16[p, col, f] = Wh[src] (fp16),
    sj_slot[p, col] = s_j[src] (f32, -1e30 at pad slots), si per tile,
    zero-degree flags.  No arithmetic on features happens on host.
  - Launch B (x2, one per GAT layer): streams the dense fp16 tables at
    line rate (plain dma_start, no gathers), computes masked segment
    softmax over the slot axis, alpha-weighted message sum (fp16
    multiply in place, f32 accumulate), leaky-relu, and the epilogue
    matmul with the next layer's weights (block-diagonal pair trick)
    -> next-layer Wh^T + attention scalars (or final fc output).
"""

import dataclasses
import numpy as np

import concourse.bacc as bacc
import concourse.tile as tile
from concourse import bass, mybir, bass_utils
from concourse.masks import make_identity

F32 = mybir.dt.float32
F16 = mybir.dt.float16

N_NODES = 100000
N_CORES = 8
DPC = N_NODES // N_CORES
F = 64
IN_C = 128
NEG_BIG = -1.0e30
ALPHA = 0.2
CALL_W = 256  # max slot-columns per call chunk
FLUSH_PAIRS = 4  # tile-pairs per epilogue matmul (512 psum cols)


@dataclasses.dataclass
class Schedule:
    n_tiles: int  # tiles per core (even)
    w_total: int  # total slot columns
    calls: list  # (t0, ntc, D, col0) ; ntc even
    gids: np.ndarray  # [N_CORES, n_tiles*128] global dst id or -1
    slot_src: np.ndarray  # [N_CORES, 128, w_total] src id or N_NODES (pad)
    si_gid: np.ndarray  # [N_CORES, 128, n_tiles] dst gid clipped (for si gather)
    flags: np.ndarray  # [N_CORES, 128, n_tiles] f32 1.0 where real dst with deg>0
    tile_col0: np.ndarray  # [n_tiles] starting col of each tile
    tile_D: np.ndarray  # [n_tiles] capacity of each tile


def build_schedule(edge_index: np.ndarray) -> Schedule:
    src = np.asarray(edge_index[0], dtype=np.int64)
    dst = np.asarray(edge_index[1], dtype=np.int64)
    order = np.argsort(dst, kind="stable")
    src_s = src[order]
    deg_all = np.bincount(dst, minlength=N_NODES).astype(np.int64)
    starts_all = np.concatenate([[0], np.cumsum(deg_all)])

    n_tiles = -(-DPC // 128)
    if n_tiles % 2:
        n_tiles += 1
    ntile_slots = n_tiles * 128

    # per-core degree-sorted dst order, padded with -1
    gids = np.full((N_CORES, ntile_slots), -1, np.int64)
    for c in range(N_CORES):
        degc = deg_all[c * DPC : (c + 1) * DPC]
        rank = np.argsort(degc, kind="stable")
        gids[c, :DPC] = c * DPC + rank

    deg_pad = np.concatenate([deg_all, [0]])
    gclip = np.where(gids >= 0, gids, N_NODES)
    degs = deg_pad[gclip].reshape(N_CORES, n_tiles, 128)
    tile_max = degs.max(axis=2).max(axis=0)  # [n_tiles] cross-core max deg

    # call plan over tile PAIRS: group pairs while ntc*D <= CALL_W
    pair_max = np.maximum(tile_max[0::2], tile_max[1::2])
    calls = []
    col = 0
    p0 = 0
    n_pairs = n_tiles // 2
    def rup4(x):
        return (int(x) + 3) // 4 * 4

    while p0 < n_pairs:
        D = max(4, rup4(pair_max[p0]))
        npair = 1
        while p0 + npair < n_pairs:
            nd = max(D, rup4(pair_max[p0 + npair]))
            if (npair + 1) * 2 * nd > CALL_W:
                break
            D = nd
            npair += 1
        calls.append((2 * p0, 2 * npair, D, col))
        col += 2 * npair * D
        p0 += npair
    w_total = col

    tile_col0 = np.zeros(n_tiles, np.int64)
    tile_D = np.zeros(n_tiles, np.int64)
    for (t0, ntc, D, col0) in calls:
        for tl in range(ntc):
            tile_col0[t0 + tl] = col0 + tl * D
            tile_D[t0 + tl] = D

    # slot_src: vectorized CSR -> padded-slot scatter
    slot_src = np.full((N_CORES, 128, w_total), N_NODES, np.int64)
    colstart_of_slot = tile_col0[
        np.arange(ntile_slots) // 128
    ]  # [ntile_slots] per (tile,partition)
    for c in range(N_CORES):
        g = gclip[c]
        ne = deg_pad[g]
        p_of_slot = np.arange(ntile_slots) % 128
        # flat positions in [128, w_total]: p*w_total + colstart + d
        base = p_of_slot * w_total + colstart_of_slot
        tot = int(ne.sum())
        pos = np.repeat(base, ne) + (
            np.arange(tot) - np.repeat(np.cumsum(ne) - ne, ne)
        )
        srcidx = np.repeat(starts_all[g], ne) + (
            np.arange(tot) - np.repeat(np.cumsum(ne) - ne, ne)
        )
        flat = slot_src[c].reshape(-1)
        flat[pos] = src_s[srcidx]

    si_gid = gclip.reshape(N_CORES, n_tiles, 128).transpose(0, 2, 1)
    flags = (
        ((gids >= 0) & (deg_pad[gclip] > 0))
        .reshape(N_CORES, n_tiles, 128)
        .transpose(0, 2, 1)
        .astype(np.float32)
    )
    flags = np.ascontiguousarray(flags)
    si_gid = np.ascontiguousarray(si_gid)

    return Schedule(
        n_tiles, w_total, calls, gids, slot_src, si_gid, flags, tile_col0, tile_D
    )


# ---------------------------------------------------------------- prog A
def build_progA(n_loc=DPC, in_c=IN_C, f=F):
    """whs[0:64] = (x W + bW)^T fp16 ; whs[64] = s_i ; whs[65] = s_j.

    Uses an augmented weight Waug = [W | W@A_i | W@A_j] (built on device)
    so each 512-column chunk is one matmul + one activation:
      x (W As) + bW As == ((x W + bW) As).
    """
    AF = mybir.ActivationFunctionType
    nc = bacc.Bacc("TRN2", target_bir_lowering=False, debug=False, num_devices=N_CORES)
    xT = nc.dram_tensor("xT", [in_c, n_loc], F16, kind="ExternalInput").ap()
    W = nc.dram_tensor("W", [in_c, f], F16, kind="ExternalInput").ap()
    bW = nc.dram_tensor("bW", [f, 1], F32, kind="ExternalInput").ap()
    As = nc.dram_tensor("As", [f, 2], F16, kind="ExternalInput").ap()
    whs = nc.dram_tensor("whs", [f + 2, n_loc], F16, kind="ExternalOutput").ap()

    CH = 512
    BATCH = 4

    with tile.TileContext(nc) as tc:
        with tc.tile_pool(name="sb", bufs=1) as pool, tc.tile_pool(
            name="ps", bufs=3, space="PSUM"
        ) as pps, tc.tile_pool(name="sb2", bufs=3) as pool2:
            xT_sb = pool.tile([in_c, n_loc], F16)
            nc.sync.dma_start(out=xT_sb[:], in_=xT[:, :])
            W_sb = pool.tile([in_c, f], F16)
            nc.sync.dma_start(out=W_sb[:], in_=W[:, :])
            bW_sb = pool.tile([f, 1], F32)
            nc.sync.dma_start(out=bW_sb[:], in_=bW[:, :])
            As_sb = pool.tile([f, 2], F16)
            nc.sync.dma_start(out=As_sb[:], in_=As[:, :])
            ident = pool.tile([128, 128], F16)
            make_identity(nc, ident[:])

            # Waug = [W | W@As] built on device
            Waug = pool.tile([in_c, f + 2], F16)
            nc.vector.tensor_copy(out=Waug[:, :f], in_=W_sb[:])
            ps_wt = pps.tile([f, 128], F16, space="PSUM", bufs=1)
            nc.tensor.transpose(out=ps_wt[:], in_=W_sb[:], identity=ident[:])
            WT_sb = pool.tile([f, 128], F16)
            nc.scalar.activation(out=WT_sb[:], in_=ps_wt[:], func=AF.Identity)
            ps_was = pps.tile([2, 128], F32, space="PSUM", bufs=1)
            nc.tensor.matmul(
                out=ps_was[:], lhsT=As_sb[:], rhs=WT_sb[:], start=True, stop=True
            )
            WAsT_sb = pool.tile([2, 128], F16)
            nc.scalar.activation(out=WAsT_sb[:], in_=ps_was[:], func=AF.Identity)
            ps_was2 = pps.tile([128, 2], F16, space="PSUM", bufs=1)
            nc.tensor.transpose(
                out=ps_was2[:], in_=WAsT_sb[:], identity=ident[:2, :2]
            )
            nc.scalar.activation(out=Waug[:, f : f + 2], in_=ps_was2[:], func=AF.Identity)

            # baug = [bW ; bW@As]
            baug = pool.tile([f + 2, 1], F32)
            nc.vector.tensor_copy(out=baug[:f], in_=bW_sb[:])
            bW16 = pool.tile([f, 1], F16)
            nc.vector.tensor_copy(out=bW16[:], in_=bW_sb[:])
            ps_bas = pps.tile([2, 1], F32, space="PSUM", bufs=1)
            nc.tensor.matmul(
                out=ps_bas[:], lhsT=As_sb[:], rhs=bW16[:], start=True, stop=True
            )
            nc.vector.tensor_copy(out=baug[f : f + 2], in_=ps_bas[:])

            for b0 in range(0, n_loc, CH * BATCH):
                bw = min(CH * BATCH, n_loc - b0)
                out_sb = pool2.tile([f + 2, CH * BATCH], F16, tag="out")
                for k, c0 in enumerate(range(b0, b0 + bw, CH)):
                    ch = min(CH, b0 + bw - c0)
                    ps_w = pps.tile([f + 2, CH], F32, tag="psw", space="PSUM")
                    nc.tensor.matmul(
                        out=ps_w[:, :ch],
                        lhsT=Waug[:],
                        rhs=xT_sb[:, c0 : c0 + ch],
                        start=True,
                        stop=True,
                    )
                    if k % 2 == 0:
                        nc.scalar.activation(
                            out=out_sb[:, c0 - b0 : c0 - b0 + ch],
                            in_=ps_w[:, :ch],
                            func=AF.Identity,
                            bias=baug[:],
                        )
                    else:
                        nc.vector.tensor_tensor(
                            out=out_sb[:, c0 - b0 : c0 - b0 + ch],
                            in0=ps_w[:, :ch],
                            in1=baug[:].to_broadcast([f + 2, ch]),
                            op=mybir.AluOpType.add,
                        )
                nc.sync.dma_start(out=whs[:, b0 : b0 + bw], in_=out_sb[:, :bw])
    nc.compile()
    return nc


# ---------------------------------------------------------------- prog B
def build_progB(sched: Schedule, f=F):
    NT = sched.n_tiles
    WTOT = sched.w_total
    NPAIR = NT // 2
    nc = bacc.Bacc("TRN2", target_bir_lowering=False, debug=False, num_devices=N_CORES)
    stream = nc.dram_tensor("stream", [128, WTOT * f], F16, kind="ExternalInput").ap()
    sj_d = nc.dram_tensor("sj", [128, WTOT], F32, kind="ExternalInput").ap()
    si_d = nc.dram_tensor("si", [128, NT], F32, kind="ExternalInput").ap()
    flags_d = nc.dram_tensor("flags", [128, NT], F32, kind="ExternalInput").ap()
    bA_d = nc.dram_tensor("bA", [128, 1], F32, kind="ExternalInput").ap()
    WnBD_d = nc.dram_tensor("WnBD", [128, 128], F16, kind="ExternalInput").ap()
    bWn_d = nc.dram_tensor("bWn", [128, 1], F32, kind="ExternalInput").ap()
    AsBD_d = nc.dram_tensor("AsBD", [128, 4], F16, kind="ExternalInput").ap()
    whnT = nc.dram_tensor("whnT", [128, NPAIR * 128], F16, kind="ExternalOutput").ap()
    sn = nc.dram_tensor("sn", [4, NPAIR * 128], F32, kind="ExternalOutput").ap()

    X = mybir.AxisListType.X
    AF = mybir.ActivationFunctionType
    OP = mybir.AluOpType
    MAXNTC = max(ntc for (_, ntc, _, _) in sched.calls)

    def v(ap, dims, off=0):
        return dataclasses.replace(
            ap,
            ap=[list(ap.ap[0])] + [list(d) for d in dims],
            offset=ap.offset + off,
        )

    with tile.TileContext(nc) as tc:
        with tc.tile_pool(name="const", bufs=1) as pc, tc.tile_pool(
            name="io", bufs=3
        ) as pio, tc.tile_pool(name="work", bufs=3) as pw, tc.tile_pool(
            name="ps", bufs=2, space="PSUM"
        ) as pps, tc.tile_pool(name="ps2", bufs=2, space="PSUM") as pps2, tc.tile_pool(
            name="ep", bufs=2
        ) as pep:
            sj_sb = pc.tile([128, WTOT], F32)
            nc.sync.dma_start(out=sj_sb[:], in_=sj_d[:, :])
            si_sb = pc.tile([128, NT], F32)
            nc.sync.dma_start(out=si_sb[:], in_=si_d[:, :])
            flags_sb = pc.tile([128, NT], F32)
            nc.sync.dma_start(out=flags_sb[:], in_=flags_d[:, :])
            bA_sb = pc.tile([128, 1], F32)
            nc.sync.dma_start(out=bA_sb[:], in_=bA_d[:, :])
            WnBD_sb = pc.tile([128, 128], F16)
            nc.sync.dma_start(out=WnBD_sb[:], in_=WnBD_d[:, :])
            bWn_sb = pc.tile([128, 1], F32)
            nc.sync.dma_start(out=bWn_sb[:], in_=bWn_d[:, :])
            AsBD_sb = pc.tile([128, 4], F16)
            nc.sync.dma_start(out=AsBD_sb[:], in_=AsBD_d[:, :])
            ident = pc.tile([128, 128], F16)
            make_identity(nc, ident[:])

            # epilogue flush state: stacked-pair h columns awaiting matmul
            state = {"hgrp": None, "k0": 0, "n": 0}

            def flush_pairs():
                if not state["n"]:
                    return
                hgrp = state["hgrp"]
                k0 = state["k0"]
                cols = state["n"] * 128
                ps_w = pps2.tile([128, FLUSH_PAIRS * 128], F32, tag="psw", space="PSUM")
                nc.tensor.matmul(
                    out=ps_w[:, :cols],
                    lhsT=WnBD_sb[:],
                    rhs=hgrp[:, :cols],
                    start=True,
                    stop=True,
                )
                whn_sb = pep.tile([128, FLUSH_PAIRS * 128], F16, tag="whn")
                nc.scalar.activation(
                    out=whn_sb[:, :cols],
                    in_=ps_w[:, :cols],
                    func=AF.Identity,
                    bias=bWn_sb[:],
                )
                nc.sync.dma_start(
                    out=whnT[:, k0 * 128 : k0 * 128 + cols], in_=whn_sb[:, :cols]
                )
                ps_s = pps2.tile([4, FLUSH_PAIRS * 128], F32, tag="pss", space="PSUM")
                nc.tensor.matmul(
                    out=ps_s[:, :cols],
                    lhsT=AsBD_sb[:],
                    rhs=whn_sb[:, :cols],
                    start=True,
                    stop=True,
                )
                s_sb = pep.tile([4, FLUSH_PAIRS * 128], F32, tag="ssb")
                nc.scalar.activation(
                    out=s_sb[:, :cols], in_=ps_s[:, :cols], func=AF.Identity
                )
                nc.sync.dma_start(
                    out=sn[:, k0 * 128 : k0 * 128 + cols], in_=s_sb[:, :cols]
                )
                state["hgrp"] = None
                state["n"] = 0

            asc = sorted(sched.calls, key=lambda cc: cc[1] * cc[2])
            order = asc[0::2] + asc[1::2][::-1]
            for (t0, ntc, D, col0) in order:
                W = ntc * D
                st = pio.tile([128, CALL_W * f], F16, tag="st")
                nc.sync.dma_start(
                    out=st[:, : W * f], in_=stream[:, col0 * f : (col0 + W) * f]
                )
                # e = leaky(s_j + s_i + bA); pads carry -1e30 inside sj
                epre = pw.tile([128, CALL_W], F32, tag="epre")
                nc.vector.tensor_tensor(
                    out=v(epre[:], [(D, ntc), (1, D)]),
                    in0=v(sj_sb[:], [(D, ntc), (1, D)], off=col0),
                    in1=si_sb[:, t0 : t0 + ntc].to_broadcast([128, ntc, D]),
                    op=OP.add,
                )
                e1 = pw.tile([128, CALL_W], F32, tag="e1")
                nc.scalar.activation(
                    out=e1[:, :W],
                    in_=epre[:, :W],
                    func=AF.Prelu,
                    bias=bA_sb[:],
                    alpha=ALPHA,
                )
                # segment softmax over slot axis. No max-subtraction: the
                # shift cancels in exp(e)/sum(exp(e)) and |e| <= ~20 here;
                # +1e-30 guards all-pad (phantom) rows against 1/0.
                ex = pw.tile([128, CALL_W], F32, tag="ex")
                nc.scalar.activation(out=ex[:, :W], in_=e1[:, :W], func=AF.Exp)
                den = pw.tile([128, MAXNTC], F32, tag="den")
                nc.vector.tensor_reduce(
                    out=den[:, :ntc],
                    in_=v(ex[:], [(D, ntc), (1, D)]),
                    axis=X,
                    op=OP.add,
                )
                dene = pw.tile([128, MAXNTC], F32, tag="dene")
                nc.vector.tensor_scalar(
                    out=dene[:, :ntc],
                    in0=den[:, :ntc],
                    scalar1=1e-30,
                    scalar2=None,
                    op0=OP.add,
                )
                rnorm = pw.tile([128, MAXNTC], F32, tag="rnorm")
                nc.vector.reciprocal(out=rnorm[:, :ntc], in_=dene[:, :ntc])
                nc.vector.tensor_tensor(
                    out=rnorm[:, :ntc],
                    in0=rnorm[:, :ntc],
                    in1=flags_sb[:, t0 : t0 + ntc],
                    op=OP.mult,
                )
                exn = pw.tile([128, CALL_W], F16, tag="exn")
                nc.vector.tensor_tensor(
                    out=v(exn[:], [(D, ntc), (1, D)]),
                    in0=v(ex[:], [(D, ntc), (1, D)]),
                    in1=rnorm[:, :ntc].to_broadcast([128, ntc, D]),
                    op=OP.mult,
                )
                # weighted messages in place over the stream tile (fp16,
                # feature-major: element (t, j, d) at offset t*f*D + j*D + d)
                nc.vector.tensor_tensor(
                    out=v(st[:], [(f * D, ntc), (D, f), (1, D)]),
                    in0=v(st[:], [(f * D, ntc), (D, f), (1, D)]),
                    in1=v(exn[:], [(D, ntc), (0, f), (1, D)]),
                    op=OP.mult,
                )
                # fold D -> D/2 with a 2x-mode tensor_tensor add (D % 4 == 0
                # so both halves stay pair-aligned), then 1x-mode reduce
                D2 = D // 2
                with nc.allow_low_precision(reason="fp16 segment sum, <=128 terms"):
                    nc.vector.tensor_tensor(
                        out=v(st[:], [(f * D, ntc), (D, f), (1, D2)]),
                        in0=v(st[:], [(f * D, ntc), (D, f), (1, D2)]),
                        in1=v(st[:], [(f * D, ntc), (D, f), (1, D2)], off=D2),
                        op=OP.add,
                    )
                    hc = pw.tile([128, MAXNTC * f], F16, tag="hc")
                    nc.vector.tensor_reduce(
                        out=hc[:, : ntc * f],
                        in_=v(st[:], [(f * D, ntc), (D, f), (1, D2)]),
                        axis=X,
                        op=OP.add,
                    )
                # epilogue per tile pair: transpose + leaky into the flush group
                for pr in range(ntc // 2):
                    kpair = (t0 + 2 * pr) // 2
                    ps_t = pps.tile([128, 128], F16, tag="pst", space="PSUM")
                    nc.tensor.transpose(
                        out=ps_t[:],
                        in_=hc[:, 2 * pr * f : (2 * pr + 2) * f],
                        identity=ident[:],
                    )
                    if state["n"] == 0:
                        state["hgrp"] = pep.tile(
                            [128, FLUSH_PAIRS * 128], F16, tag="hgrp", name="hgrp"
                        )
                        state["k0"] = kpair
                    j = state["n"]
                    nc.scalar.activation(
                        out=state["hgrp"][:, j * 128 : (j + 1) * 128],
                        in_=ps_t[:],
                        func=AF.Prelu,
                        alpha=ALPHA,
                    )
                    state["n"] += 1
                    if state["n"] == FLUSH_PAIRS:
                        flush_pairs()
                flush_pairs()
    nc.compile()
    return nc


# ---------------------------------------------------------------- driver
_cache = {}


def kernel(x, edge_index, W1, bW1, A1, bA1, W2, bW2, A2, bA2, Wfc, bfc):
    x = np.asarray(x, dtype=np.float32)
    edge_index = np.asarray(edge_index)
    W1 = np.asarray(W1, np.float32)
    bW1 = np.asarray(bW1, np.float32)
    A1 = np.asarray(A1, np.float32)
    bA1 = np.asarray(bA1, np.float32)
    W2 = np.asarray(W2, np.float32)
    bW2 = np.asarray(bW2, np.float32)
    A2 = np.asarray(A2, np.float32)
    bA2 = np.asarray(bA2, np.float32)
    Wfc = np.asarray(Wfc, np.float32)
    bfc = np.asarray(bfc, np.float32)

    sched = build_schedule(edge_index)
    cores = list(range(N_CORES))
    NT = sched.n_tiles
    NPAIR = NT // 2

    if "A" not in _cache:
        _cache["A"] = build_progA()
    ncA = _cache["A"]
    inA = []
    x16T = np.ascontiguousarray(x.T.astype(np.float16))
    W1_16 = W1.astype(np.float16)
    As1_16 = np.ascontiguousarray(
        np.concatenate([A1[:F], A1[F:]], axis=1).astype(np.float16)
    )
    for c in cores:
        inA.append(
            {
                "xT": np.ascontiguousarray(x16T[:, c * DPC : (c + 1) * DPC]),
                "W": W1_16,
                "bW": bW1.reshape(F, 1),
                "As": As1_16,
            }
        )
    resA = bass_utils.run_bass_kernel_spmd(ncA, inA, core_ids=cores)
    whs = np.concatenate([resA.results[c]["whs"] for c in cores], axis=1)
    wh = np.ascontiguousarray(whs[:F].T)
    si_full = whs[F].astype(np.float32)
    sj_full = whs[F + 1].astype(np.float32)

    key = ("B", NT, sched.w_total, tuple(sched.calls))
    if key not in _cache:
        _cache[key] = build_progB(sched)
    ncB = _cache[key]

    def launch_B(wh_full, si_f, sj_f, bA, Wn, bWn, An):
        wh16 = np.concatenate(
            [wh_full.astype(np.float16), np.zeros((1, F), np.float16)], axis=0
        )
        sjpad = np.concatenate([sj_f, [np.float32(NEG_BIG)]]).astype(np.float32)
        sipad = np.concatenate([si_f, [np.float32(0.0)]]).astype(np.float32)
        WnBD = np.zeros((128, 128), np.float16)
        WnBD[:F, :F] = Wn
        WnBD[F:, F:] = Wn
        AsBD = np.zeros((128, 4), np.float16)
        AsBD[:F, 0:1] = An[:, 0:1]
        AsBD[:F, 1:2] = An[:, 1:2]
        AsBD[F:, 2:3] = An[:, 0:1]
        AsBD[F:, 3:4] = An[:, 1:2]
        bWn2 = np.concatenate([bWn.reshape(F), bWn.reshape(F)]).reshape(128, 1)
        inB = []
        for c in cores:
            ss = sched.slot_src[c]
            # feature-major stream: per call, element (t, j, d) at t*F*D+j*D+d
            stream = np.empty((128, sched.w_total * F), np.float16)
            asc = sorted(sched.calls, key=lambda cc: cc[1] * cc[2])
            order = asc[0::2] + asc[1::2][::-1]
            for (t0, ntc, D, col0) in order:
                W = ntc * D
                blk = wh16[ss[:, col0 : col0 + W]].reshape(128, ntc, D, F)
                stream[:, col0 * F : (col0 + W) * F] = (
                    blk.transpose(0, 1, 3, 2).reshape(128, W * F)
                )
            inB.append(
                {
                    "stream": stream,
                    "sj": sjpad[ss],
                    "si": sipad[sched.si_gid[c]],
                    "flags": sched.flags[c],
                    "bA": np.full((128, 1), bA.reshape(-1)[0], np.float32),
                    "WnBD": WnBD,
                    "bWn": bWn2,
                    "AsBD": AsBD,
                }
            )
        res = bass_utils.run_bass_kernel_spmd(ncB, inB, core_ids=cores)
        whn = np.zeros((N_NODES, F), np.float32)
        sn_i = np.zeros(N_NODES, np.float32)
        sn_j = np.zeros(N_NODES, np.float32)
        for c in cores:
            gids = sched.gids[c]
            real = gids >= 0
            w = res.results[c]["whnT"].astype(np.float32).reshape(128, NPAIR, 128)
            snc = res.results[c]["sn"].reshape(4, NPAIR, 128)
            # tile 2k -> rows 0:64 of pair k; tile 2k+1 -> rows 64:128
            wA = w[:F].transpose(1, 2, 0)  # [NPAIR, 128, F] even tiles
            wB = w[F:].transpose(1, 2, 0)  # odd tiles
            wfull = np.empty((NT, 128, F), np.float32)
            wfull[0::2] = wA
            wfull[1::2] = wB
            sfull_i = np.empty((NT, 128), np.float32)
            sfull_j = np.empty((NT, 128), np.float32)
            sfull_i[0::2] = snc[0]
            sfull_i[1::2] = snc[2]
            sfull_j[0::2] = snc[1]
            sfull_j[1::2] = snc[3]
            whn[gids[real]] = wfull.reshape(NT * 128, F)[real]
            sn_i[gids[real]] = sfull_i.reshape(-1)[real]
            sn_j[gids[real]] = sfull_j.reshape(-1)[real]
        return whn, sn_i, sn_j

    As2 = np.ascontiguousarray(np.concatenate([A2[:F], A2[F:]], axis=1))
    wh2, si2, sj2 = launch_B(wh, si_full, sj_full, bA1, W2, bW2, As2)
    out, _, _ = launch_B(wh2, si2, sj2, bA2, Wfc, bfc, np.zeros((F, 2), np.float32))
    return out.astype(np.float32)
